# revision 9
# baseline (speedup 1.0000x reference)
"""CosinePrediction edge-parallel kernel for 8 trn2 NeuronCores — v2.

Strategy (replaces SWDGE dma_gather, which was descriptor-rate-bound at
~8.5ns/desc/queue): edges are sharded by SRC RANGE (core c owns src rows
[c*12500, (c+1)*12500)). Host transposes both feature tables; each core keeps
its uT slice [64, 12500] f32 resident in SBUF and streams hiT in 16 blocks of
6250 rows, each written to partitions 0-63 (lane A) and duplicated to 64-127
(lane B). Per 4096-edge call, ONE GPSIMD ap_gather with interleaved index
lists (T src idxs, then 12500+parity*6250+dst idxs, per 16-partition group)
pulls u and v feature columns for two 2048-edge tiles at once. DVE forms
m = u*v and v^2, ACT forms u^2; PE reduces the 64-partition feature dim with
a [128,2] lane-indicator stationary into PSUM [2,512] chunks; the narrow tail
computes cos = m_sum / sqrt(uu*vv) and DMAs straight out.
"""
import sys
import os

sys.path.insert(0, "/opt/trn_rl_repo")

import numpy as np
from contextlib import ExitStack

N = 100_000            # rows per table
D = 64                 # feature dim
E_TOTAL = 1_000_000
NCORES = 8
USLICE = N // NCORES   # 12500 u rows per core
NBLK = 16              # v blocks per core
VBLK = N // NBLK       # 6250 v rows per block
T = 2048               # edges per tile (one lane)
CALLS = 32             # ap_gather calls per core (2 per block)
CAP_LANE_BLK = 2 * T   # 4096 padded edges per (lane, block)
CAP_CORE = 2 * NBLK * CAP_LANE_BLK  # 131072 padded edges per core
NELEMS = USLICE + 2 * VBLK          # 25000 table columns (u + 2 v parities)
C = 1024               # psum chunk (edges)
CHUNKS = T // C        # 2 chunks per call
IDXC = 2 * T // 16     # 256 idx columns per call per partition

LAST_RESULT = None
_CACHED_NC = None


def _install_trace_shim():
    try:
        import types
        if "antenv.axon_hooks" not in sys.modules:
            from trn_agent_boot.trn_boot import _ntff_profile_via_ctypes
            hook = _ntff_profile_via_ctypes("/opt/axon/libaxon_pjrt.so")
            mod = types.ModuleType("antenv.axon_hooks")
            mod.get_axon_ntff_profile_hook = lambda: hook
            mod.set_axon_ntff_profile_hook = lambda h: None
            sys.modules["antenv.axon_hooks"] = mod
            import antenv
            antenv.axon_hooks = mod
        import concourse.bass_utils as bu
        bu.upload_artifacts = lambda tmpdir: f"file://{tmpdir}"
    except Exception:
        pass


def build_nc():
    global _CACHED_NC
    if _CACHED_NC is not None:
        return _CACHED_NC
    import concourse.bass as bass
    import concourse.bacc as bacc
    import concourse.mybir as mybir
    from concourse.library_config import ap_gather as apg_lib

    f32 = mybir.dt.float32
    bf16 = mybir.dt.bfloat16
    i16 = mybir.dt.int16
    MUL = mybir.AluOpType.mult
    DIV = mybir.AluOpType.divide
    SQ = mybir.ActivationFunctionType.Square
    SQRT = mybir.ActivationFunctionType.Sqrt
    CP = mybir.ActivationFunctionType.Copy

    nc = bacc.Bacc("TRN2", target_bir_lowering=False, debug=False,
                   detect_race_conditions=False)
    hut = nc.dram_tensor("hut", [D, USLICE], f32, kind="ExternalInput")
    hit = nc.dram_tensor("hit", [D, N], f32, kind="ExternalInput")
    idx_d = nc.dram_tensor("idx", [128, CALLS * IDXC], i16, kind="ExternalInput")
    ones_d = nc.dram_tensor("ones", [128, 2], bf16, kind="ExternalInput")
    out_d = nc.dram_tensor("out", [2, CALLS * T], f32, kind="ExternalOutput")

    with ExitStack() as st:
        tables = st.enter_context(nc.sbuf_tensor("tables", [128, NELEMS], f32))
        idx = st.enter_context(nc.sbuf_tensor("idx_sb", [128, CALLS * IDXC], i16))
        ones = st.enter_context(nc.sbuf_tensor("ones_sb", [128, 2], bf16))
        gout = [st.enter_context(nc.sbuf_tensor(f"gout{s}", [128, 2 * T], f32))
                for s in range(2)]
        m = [st.enter_context(nc.sbuf_tensor(f"m{s}", [128, T], bf16))
             for s in range(2)]
        squ = [st.enter_context(nc.sbuf_tensor(f"squ{s}", [128, T], bf16))
               for s in range(2)]
        sqv = [st.enter_context(nc.sbuf_tensor(f"sqv{s}", [128, T], bf16))
               for s in range(2)]
        d2 = [st.enter_context(nc.sbuf_tensor(f"d2_{s}", [2, C], f32))
              for s in range(2)]
        outc = [st.enter_context(nc.sbuf_tensor(f"outc{s}", [2, C], f32))
                for s in range(4)]
        vv_s = [st.enter_context(nc.sbuf_tensor(f"vv_s{s}", [2, C], f32))
                for s in range(2)]
        mp = st.enter_context(nc.psum_tensor("mp", [66, C], f32))
        up = st.enter_context(nc.psum_tensor("up", [66, C], f32))
        vp = st.enter_context(nc.psum_tensor("vp", [66, C], f32))

        S_i = st.enter_context(nc.semaphore("S_i"))      # idx dma
        S_u = st.enter_context(nc.semaphore("S_u"))      # u hbm load
        S_u2 = st.enter_context(nc.semaphore("S_u2"))    # u dup
        S_on = st.enter_context(nc.semaphore("S_on"))    # ones dma
        S_vh = st.enter_context(nc.semaphore("S_vh"))    # v hbm load (16/blk)
        S_tab = st.enter_context(nc.semaphore("S_tab"))  # v dup done (16/blk)
        S_g = st.enter_context(nc.semaphore("S_g"))      # gathers done (1/call)
        S_m = st.enter_context(nc.semaphore("S_m"))      # m-mult done (1/call)
        S_squ = st.enter_context(nc.semaphore("S_squ"))  # u^2 done (1/call)
        S_sqv = st.enter_context(nc.semaphore("S_sqv"))  # v^2 done (1/call)
        S_pe = st.enter_context(nc.semaphore("S_pe"))    # matmuls (1 each)
        S_vv = st.enter_context(nc.semaphore("S_vv"))    # vp->sbuf copy
        S_d2 = st.enter_context(nc.semaphore("S_d2"))    # d2 per chunk
        S_den = st.enter_context(nc.semaphore("S_den"))  # sqrt per chunk
        S_f = st.enter_context(nc.semaphore("S_f"))      # final per chunk
        S_od = st.enter_context(nc.semaphore("S_od"))    # out dma (16/chunk)

        block = st.enter_context(nc.Block())

        def voff(b):
            return USLICE + (b % 2) * VBLK

        @block.sync
        def _(sync):
            sync.dma_start(idx[:], idx_d[:, :]).then_inc(S_i, 16)
            sync.dma_start(ones[:], ones_d[:, :]).then_inc(S_on, 16)
            sync.dma_start(tables[0:64, 0:USLICE], hut[:, :]).then_inc(S_u, 16)
            sync.wait_ge(S_u, 16)
            sync.dma_start(tables[64:128, 0:USLICE],
                           tables[0:64, 0:USLICE]).then_inc(S_u2, 16)
            for b in range(NBLK):
                # drain finished chunks of calls 2b-4, 2b-3
                if b >= 2:
                    for k in (2 * b - 4, 2 * b - 3):
                        for j in range(CHUNKS):
                            jg = CHUNKS * k + j
                            sync.wait_ge(S_f, jg + 1)
                            sync.dma_start(
                                out_d[0:2, k * T + j * C:k * T + (j + 1) * C],
                                outc[jg % 4][:, :]).then_inc(S_od, 16)
                # v block b -> lane A, then dup to lane B
                if b >= 1:
                    sync.wait_ge(S_g, max(0, 2 * (b - 1)))
                vo = voff(b)
                sync.dma_start(tables[0:64, vo:vo + VBLK],
                               hit[:, b * VBLK:(b + 1) * VBLK]
                               ).then_inc(S_vh, 16)
                sync.wait_ge(S_vh, 16 * (b + 1))
                sync.dma_start(tables[64:128, vo:vo + VBLK],
                               tables[0:64, vo:vo + VBLK]).then_inc(S_tab, 16)
            for k in (CALLS - 4, CALLS - 3, CALLS - 2, CALLS - 1):
                for j in range(CHUNKS):
                    jg = CHUNKS * k + j
                    sync.wait_ge(S_f, jg + 1)
                    sync.dma_start(
                        out_d[0:2, k * T + j * C:k * T + (j + 1) * C],
                        outc[jg % 4][:, :]).then_inc(S_od, 16)
            sync.wait_ge(S_od, 16 * CALLS * CHUNKS)

        @block.gpsimd
        def _(gpsimd):
            gpsimd.load_library(apg_lib)
            gpsimd.wait_ge(S_i, 16)
            gpsimd.wait_ge(S_u2, 16)
            for k in range(CALLS):
                b = k // 2
                if k >= 2:
                    gpsimd.wait_ge(S_m, k - 1)
                    gpsimd.wait_ge(S_squ, k - 1)
                    gpsimd.wait_ge(S_sqv, k - 1)
                gpsimd.wait_ge(S_tab, 16 * (b + 1))
                gpsimd.ap_gather(
                    gout[k % 2][:, :], tables[:, :],
                    idx[:, k * IDXC:(k + 1) * IDXC],
                    128, NELEMS, 1, 2 * T,
                ).then_inc(S_g, 1)

        def narrow_dve(vector, k):
            for j in range(CHUNKS):
                jg = CHUNKS * k + j
                po = 32 * (jg % 3)
                vector.wait_ge(S_pe, 12 * k + 6 * j + 5)
                vector.wait_ge(S_vv, jg + 1)
                vector.tensor_tensor(out=d2[jg % 2][:],
                                     in0=up[po:po + 2, :],
                                     in1=vv_s[jg % 2][:], op=MUL
                                     ).then_inc(S_d2, 1)
                vector.wait_ge(S_den, jg + 1)
                vector.reciprocal(out=vv_s[jg % 2][:], in_=d2[jg % 2][:])
                if jg >= 4:
                    vector.wait_ge(S_od, 16 * (jg - 3))
                vector.tensor_tensor(out=outc[jg % 4][:],
                                     in0=mp[po:po + 2, :],
                                     in1=vv_s[jg % 2][:], op=MUL
                                     ).then_inc(S_f, 1)

        @block.vector
        def _(vector):
            import concourse.mybir as mybir
            for k in range(CALLS):
                vector.wait_ge(S_g, k + 1)
                vector.tensor_tensor(out=m[k % 2][:],
                                     in0=gout[k % 2][:, 0:T],
                                     in1=gout[k % 2][:, T:2 * T],
                                     op=MUL).then_inc(S_m, 1)
                vector.tensor_tensor(out=sqv[k % 2][:],
                                     in0=gout[k % 2][:, T:2 * T],
                                     in1=gout[k % 2][:, T:2 * T],
                                     op=MUL).then_inc(S_sqv, 1)
                if k >= 1:
                    narrow_dve(vector, k - 1)
            narrow_dve(vector, CALLS - 1)

        def narrow_act(scalar, k):
            for j in range(CHUNKS):
                jg = CHUNKS * k + j
                po = 32 * (jg % 3)
                scalar.wait_ge(S_pe, 12 * k + 6 * j + 6)
                if jg >= 2:
                    scalar.wait_ge(S_f, jg - 1)
                scalar.activation(out=vv_s[jg % 2][:], in_=vp[po:po + 2, :],
                                  func=CP).then_inc(S_vv, 1)
                scalar.wait_ge(S_d2, jg + 1)
                scalar.activation(out=d2[jg % 2][:], in_=d2[jg % 2][:],
                                  func=SQRT).then_inc(S_den, 1)

        @block.scalar
        def _(scalar):
            for k in range(CALLS):
                scalar.wait_ge(S_g, k + 1)
                scalar.activation(out=squ[k % 2][:], in_=gout[k % 2][:, 0:T],
                                  func=SQ).then_inc(S_squ, 1)
                if k >= 1:
                    narrow_act(scalar, k - 1)
            narrow_act(scalar, CALLS - 1)

        @block.tensor
        def _(tensor):
            tensor.wait_ge(S_on, 16)
            for k in range(CALLS):
                tensor.wait_ge(S_m, k + 1)
                tensor.wait_ge(S_squ, k + 1)
                tensor.wait_ge(S_sqv, k + 1)
                for j in range(CHUNKS):
                    jg = CHUNKS * k + j
                    po = 32 * (jg % 3)
                    if jg >= 3:
                        tensor.wait_ge(S_f, jg - 2)
                    for s in range(2):
                        mc = slice(j * C + s * 512, j * C + (s + 1) * 512)
                        pc = slice(s * 512, (s + 1) * 512)
                        tensor.matmul(mp[po:po + 2, pc], ones[:, :],
                                      m[k % 2][:, mc]).then_inc(S_pe, 1)
                        tensor.matmul(up[po:po + 2, pc], ones[:, :],
                                      squ[k % 2][:, mc]).then_inc(S_pe, 1)
                        tensor.matmul(vp[po:po + 2, pc], ones[:, :],
                                      sqv[k % 2][:, mc]).then_inc(S_pe, 1)

    nc.compile()
    _CACHED_NC = nc
    return nc


def _prep_core(src_l, dst):
    """Build one core's idx tensor + output mapping.

    src_l: local src row ids [n] in [0, USLICE); dst: global dst ids [n].
    Returns (idx [128, CALLS*IDXC] int16, lane [n], pos [n]) where the edge's
    cosine lands at out[lane, pos]."""
    n = len(src_l)
    assert n <= CAP_CORE, n
    blk = dst // VBLK
    order = np.argsort(blk, kind="stable")
    counts = np.bincount(blk, minlength=NBLK)
    assert counts.max() <= CAP_LANE_BLK * 2, counts.max()
    starts = np.zeros(NBLK, np.int64)
    starts[1:] = np.cumsum(counts)[:-1]
    # rank within block, in sorted order
    rank_sorted = np.arange(n) - starts[blk[order]]
    lane_s = (rank_sorted % 2).astype(np.int64)
    slot_s = rank_sorted // 2
    pos_s = blk[order] * CAP_LANE_BLK + slot_s
    lane = np.empty(n, np.int64)
    pos = np.empty(n, np.int64)
    lane[order] = lane_s
    pos[order] = pos_s

    idx = np.zeros((128, CALLS * IDXC), np.int16)
    su = src_l[order]
    du = dst[order] - blk[order] * VBLK
    for b in range(NBLK):
        s0, nb = starts[b], counts[b]
        vo = USLICE + (b % 2) * VBLK
        for t in range(2):           # tile within block
            k = 2 * b + t
            for ln in range(2):      # lane
                sel = (lane_s[s0:s0 + nb] == ln) & \
                      (slot_s[s0:s0 + nb] >= t * T) & \
                      (slot_s[s0:s0 + nb] < (t + 1) * T)
                es = s0 + np.nonzero(sel)[0]
                cnt = len(es)
                a = np.zeros(2 * T, np.int64)
                a[:cnt] = su[es]
                a[T:T + cnt] = vo + du[es]
                a[T + cnt:] = vo
                w = a.reshape(IDXC, 16).T.astype(np.int16)  # s -> (s%16, s//16)
                idx[64 * ln:64 * (ln + 1), k * IDXC:(k + 1) * IDXC] = \
                    np.tile(w, (4, 1))
    return idx, lane, pos


def kernel(h_user, h_item, src_idx, dst_idx):
    global LAST_RESULT
    from concourse.bass_utils import run_bass_kernel_spmd

    if os.environ.get("BASS_TRACE"):
        _install_trace_shim()

    hu = np.asarray(h_user, dtype=np.float32)
    hi = np.asarray(h_item, dtype=np.float32)
    src = np.asarray(src_idx).astype(np.int64)
    dst = np.asarray(dst_idx).astype(np.int64)

    huT = np.ascontiguousarray(hu.T)     # [64, 100000]
    hiT = np.ascontiguousarray(hi.T)

    ones = np.zeros((128, 2), np.float32)
    ones[0:64, 0] = 1.0
    ones[64:128, 1] = 1.0
    import ml_dtypes
    ones = ones.astype(ml_dtypes.bfloat16)

    nc = build_nc()

    core = src // USLICE
    in_maps, metas = [], []
    for c in range(NCORES):
        esel = np.nonzero(core == c)[0]
        s_l = src[esel] - c * USLICE
        d_g = dst[esel]
        idx, lane, pos = _prep_core(s_l, d_g)
        hutc = np.ascontiguousarray(huT[:, c * USLICE:(c + 1) * USLICE])
        in_maps.append({"hut": hutc, "hit": hiT, "idx": idx, "ones": ones})
        metas.append((esel, lane, pos))

    res = run_bass_kernel_spmd(nc, in_maps, core_ids=list(range(NCORES)))
    LAST_RESULT = res

    out = np.empty((E_TOTAL, 1), np.float32)
    for c in range(NCORES):
        esel, lane, pos = metas[c]
        arr = res.results[c]["out"]          # [2, CALLS*T]
        out[esel, 0] = arr[lane, pos]
    return out


# revision 12
# speedup vs baseline: 5.2198x; 5.2198x over previous
"""CosinePrediction edge-parallel kernel for 8 trn2 NeuronCores.

Strategy: shard the 1M edges across 8 cores (125k each). Per core, bucket
edges host-side by (src_quarter, dst_quarter) of the 100k-row tables —
16 buckets, capacity 8192, padded with index 0 — so row indices fit the
int16 index format of GPSIMD dma_gather. Each bucket does two dma_gather
ops (raw f32 rows of both tables, 4 SWDGE queues round-robin, double
buffered), then on-chip: s_uv = reduce(u*v), s_uu = reduce(u^2) (ACT
square + DVE reduce), s_vv likewise, and a tail computes
cos = s_uv * rsqrt(s_uu*s_vv). The host un-permutes bucket-sorted results.
"""
import sys
import os

sys.path.insert(0, "/opt/trn_rl_repo")

import numpy as np
from contextlib import ExitStack

N = 100_000          # rows per table
D = 64               # feature dim (256B rows)
E_TOTAL = 1_000_000
NCORES = 8
PER = E_TOTAL // NCORES     # 125000 edges per core
NQUART = 4
Q = N // NQUART             # 25000 rows per quarter (< int16 max)
NB = NQUART * NQUART        # 16 buckets per core
CAP = 8192                  # bucket capacity = 128 * 64 (max observed
                            # bucket count is 8019; raises on overflow)
BLK = CAP // 128            # 64 dst blocks
IDXC = CAP // 16            # 512 idx columns (wrapped in 16 partitions)
COLS = NB * BLK             # 1024 result columns

LAST_RESULT = None
_CACHED_NC = None


def _install_trace_shim():
    """Register the NTFF profile hook trn_boot couldn't (stub antenv), and
    neuter the S3 artifact upload. Only needed when BASS_TRACE=1."""
    try:
        import types
        if "antenv.axon_hooks" not in sys.modules:
            from trn_agent_boot.trn_boot import _ntff_profile_via_ctypes
            hook = _ntff_profile_via_ctypes("/opt/axon/libaxon_pjrt.so")
            mod = types.ModuleType("antenv.axon_hooks")
            mod.get_axon_ntff_profile_hook = lambda: hook
            mod.set_axon_ntff_profile_hook = lambda h: None
            sys.modules["antenv.axon_hooks"] = mod
            import antenv
            antenv.axon_hooks = mod
        import concourse.bass_utils as bu
        bu.upload_artifacts = lambda tmpdir: f"file://{tmpdir}"
    except Exception:
        pass


def build_nc():
    global _CACHED_NC
    if _CACHED_NC is not None:
        return _CACHED_NC
    import concourse.bass as bass
    import concourse.bacc as bacc
    import concourse.mybir as mybir
    from concourse.library_config import mlp

    f32 = mybir.dt.float32
    i16 = mybir.dt.int16

    nc = bacc.Bacc("TRN2", target_bir_lowering=False, debug=False,
                   num_swdge_queues=4, detect_race_conditions=False)
    hu = nc.dram_tensor("hu", [N, D], f32, kind="ExternalInput")
    hi = nc.dram_tensor("hi", [N, D], f32, kind="ExternalInput")
    idxu_d = nc.dram_tensor("idxu", [128, NB * IDXC], i16, kind="ExternalInput")
    idxv_d = nc.dram_tensor("idxv", [128, NB * IDXC], i16, kind="ExternalInput")
    out_d = nc.dram_tensor("out", [128, COLS], f32, kind="ExternalOutput")

    with ExitStack() as st:
        u = [st.enter_context(nc.sbuf_tensor(f"u{s}", [128, BLK, D], f32))
             for s in range(2)]
        v = [st.enter_context(nc.sbuf_tensor(f"v{s}", [128, BLK, D], f32))
             for s in range(2)]
        m = [st.enter_context(nc.sbuf_tensor(f"m{s}", [128, BLK, D], f32))
             for s in range(2)]
        w = [st.enter_context(nc.sbuf_tensor(f"w{s}", [128, BLK, D], f32))
             for s in range(2)]
        idxu = st.enter_context(nc.sbuf_tensor("idxu_sb", [128, NB * IDXC], i16))
        idxv = st.enter_context(nc.sbuf_tensor("idxv_sb", [128, NB * IDXC], i16))
        suv = st.enter_context(nc.sbuf_tensor("suv", [128, COLS], f32))
        suu = st.enter_context(nc.sbuf_tensor("suu", [128, COLS], f32))
        svv = st.enter_context(nc.sbuf_tensor("svv", [128, COLS], f32))

        S_idx = st.enter_context(nc.semaphore("S_idx"))
        S_q = [st.enter_context(nc.semaphore(f"S_q{q}")) for q in range(4)]
        S_suv = [st.enter_context(nc.semaphore(f"S_suv{s}")) for s in range(2)]
        S_usq = [st.enter_context(nc.semaphore(f"S_usq{s}")) for s in range(2)]
        S_vsq = [st.enter_context(nc.semaphore(f"S_vsq{s}")) for s in range(2)]
        S_mult = [st.enter_context(nc.semaphore(f"S_mult{s}")) for s in range(2)]
        S_suu = [st.enter_context(nc.semaphore(f"S_suu{s}")) for s in range(2)]
        S_t = st.enter_context(nc.semaphore("S_t"))
        S_sq = st.enter_context(nc.semaphore("S_sq"))
        S_res = st.enter_context(nc.semaphore("S_res"))
        S_out = st.enter_context(nc.semaphore("S_out"))

        block = st.enter_context(nc.Block())

        @block.sync
        def _(sync):
            sync.dma_start(idxu[:, :IDXC], idxu_d[:, :IDXC]).then_inc(S_idx, 16)
            sync.dma_start(idxv[:, :IDXC], idxv_d[:, :IDXC]).then_inc(S_idx, 16)
            sync.dma_start(idxu[:, IDXC:], idxu_d[:, IDXC:]).then_inc(S_idx, 16)
            sync.dma_start(idxv[:, IDXC:], idxv_d[:, IDXC:]).then_inc(S_idx, 16)
            sync.wait_ge(S_res, 1)
            sync.dma_start(out_d[:, :], suv[:]).then_inc(S_out, 16)
            sync.wait_ge(S_out, 16)

        @block.gpsimd
        def _(gpsimd):
            gpsimd.load_library(mlp)
            gpsimd.wait_ge(S_idx, 32)
            for b in range(NB):
                if b == 1:
                    gpsimd.wait_ge(S_idx, 64)
                s, k = b % 2, b // 2
                qs, qd = b // NQUART, b % NQUART
                H, HI = CAP // 2, IDXC // 2
                if k >= 1:
                    gpsimd.wait_ge(S_mult[s], k)
                    gpsimd.wait_ge(S_usq[s], k)
                gpsimd.dma_gather(
                    u[s][:, :BLK // 2, :], hu[qs * Q:(qs + 1) * Q, :],
                    idxu[:, b * IDXC:b * IDXC + HI],
                    H, H, D, single_packet=False, queue_num=0,
                ).then_inc(S_q[0], 16)
                gpsimd.dma_gather(
                    u[s][:, BLK // 2:, :], hu[qs * Q:(qs + 1) * Q, :],
                    idxu[:, b * IDXC + HI:(b + 1) * IDXC],
                    H, H, D, single_packet=False, queue_num=1,
                ).then_inc(S_q[1], 16)
                if k >= 1:
                    gpsimd.wait_ge(S_vsq[s], k)
                gpsimd.dma_gather(
                    v[s][:, :BLK // 2, :], hi[qd * Q:(qd + 1) * Q, :],
                    idxv[:, b * IDXC:b * IDXC + HI],
                    H, H, D, single_packet=False, queue_num=2,
                ).then_inc(S_q[2], 16)
                gpsimd.dma_gather(
                    v[s][:, BLK // 2:, :], hi[qd * Q:(qd + 1) * Q, :],
                    idxv[:, b * IDXC + HI:(b + 1) * IDXC],
                    H, H, D, single_packet=False, queue_num=3,
                ).then_inc(S_q[3], 16)

        @block.vector
        def _(vector):
            for b in range(NB):
                s, k = b % 2, b // 2
                cols = slice(b * BLK, (b + 1) * BLK)
                for q in range(4):
                    vector.wait_ge(S_q[q], 16 * (b + 1))
                vector.tensor_tensor(out=m[s][:], in0=u[s][:], in1=v[s][:],
                                     op=mybir.AluOpType.mult
                                     ).then_inc(S_mult[s], 1)
                vector.tensor_reduce(out=suv[:, cols], in_=m[s][:],
                                     axis=mybir.AxisListType.X,
                                     op=mybir.AluOpType.add,
                                     ).then_inc(S_suv[s], 1)
                vector.wait_ge(S_usq[s], k + 1)
                vector.tensor_reduce(out=suu[:, cols], in_=w[s][:],
                                     axis=mybir.AxisListType.X,
                                     op=mybir.AluOpType.add,
                                     ).then_inc(S_suu[s], 1)
                vector.wait_ge(S_vsq[s], k + 1)
                vector.tensor_reduce(out=svv[:, cols], in_=m[s][:],
                                     axis=mybir.AxisListType.X,
                                     op=mybir.AluOpType.add)
            # tail: cos = s_uv * rsqrt(s_uu * s_vv)
            vector.tensor_tensor(out=suu[:], in0=suu[:], in1=svv[:],
                                 op=mybir.AluOpType.mult).then_inc(S_t, 1)
            vector.wait_ge(S_sq, 1)
            vector.reciprocal(out=suu[:], in_=svv[:])
            vector.tensor_tensor(out=suv[:], in0=suv[:], in1=suu[:],
                                 op=mybir.AluOpType.mult).then_inc(S_res, 1)

        @block.scalar
        def _(scalar):
            for b in range(NB):
                s, k = b % 2, b // 2
                scalar.wait_ge(S_q[0], 16 * (b + 1))
                scalar.wait_ge(S_q[1], 16 * (b + 1))
                if k >= 1:
                    scalar.wait_ge(S_suu[s], k)
                scalar.activation(out=w[s][:], in_=u[s][:],
                                  func=mybir.ActivationFunctionType.Square,
                                  ).then_inc(S_usq[s], 1)
                scalar.wait_ge(S_suv[s], k + 1)
                scalar.activation(out=m[s][:], in_=v[s][:],
                                  func=mybir.ActivationFunctionType.Square,
                                  ).then_inc(S_vsq[s], 1)
            scalar.wait_ge(S_t, 1)
            scalar.activation(out=svv[:], in_=suu[:],
                              func=mybir.ActivationFunctionType.Sqrt,
                              ).then_inc(S_sq, 1)

    nc.compile()
    _CACHED_NC = nc
    return nc


def _prep_core(src, dst):
    """Bucket one core's edges; returns (idxu, idxv, order, counts)."""
    qs = src // Q
    qd = dst // Q
    bucket = qs * NQUART + qd
    order = np.argsort(bucket, kind="stable")
    counts = np.bincount(bucket, minlength=NB)
    if counts.max() > CAP:
        raise RuntimeError(f"bucket overflow: {counts.max()} > {CAP}")
    su, du = src[order], dst[order]
    idxu = np.zeros((128, NB * IDXC), np.int16)
    idxv = np.zeros((128, NB * IDXC), np.int16)
    off = 0
    for b in range(NB):
        n = counts[b]
        lu = np.zeros(CAP, np.int64)
        lv = np.zeros(CAP, np.int64)
        lu[:n] = su[off:off + n] - (b // NQUART) * Q
        lv[:n] = du[off:off + n] - (b % NQUART) * Q
        off += n
        wu = lu.reshape(IDXC, 16).T.astype(np.int16)   # i -> (i%16, i//16)
        wv = lv.reshape(IDXC, 16).T.astype(np.int16)
        idxu[:, b * IDXC:(b + 1) * IDXC] = np.tile(wu, (8, 1))
        idxv[:, b * IDXC:(b + 1) * IDXC] = np.tile(wv, (8, 1))
    return idxu, idxv, order, counts


def kernel(h_user, h_item, src_idx, dst_idx):
    global LAST_RESULT
    from concourse.bass_utils import run_bass_kernel_spmd

    if os.environ.get("BASS_TRACE"):
        _install_trace_shim()

    hu = np.ascontiguousarray(np.asarray(h_user, dtype=np.float32))
    hi = np.ascontiguousarray(np.asarray(h_item, dtype=np.float32))
    src = np.asarray(src_idx).astype(np.int64)
    dst = np.asarray(dst_idx).astype(np.int64)
    idx_dtype = np.asarray(src_idx).dtype

    nc = build_nc()

    in_maps, metas = [], []
    for c in range(NCORES):
        s = src[c * PER:(c + 1) * PER]
        d = dst[c * PER:(c + 1) * PER]
        idxu, idxv, order, counts = _prep_core(s, d)
        in_maps.append({"hu": hu, "hi": hi, "idxu": idxu, "idxv": idxv})
        metas.append((order, counts))

    res = run_bass_kernel_spmd(nc, in_maps, core_ids=list(range(NCORES)))
    LAST_RESULT = res

    outs = []
    for c in range(NCORES):
        order, counts = metas[c]
        arr = res.results[c]["out"].reshape(128, NB, BLK)
        # slot i of bucket b lives at [i % 128, b, i // 128]
        arr2 = arr.transpose(1, 2, 0).reshape(NB, CAP)
        cos_sorted = np.concatenate(
            [arr2[b, :counts[b]] for b in range(NB)])
        res_core = np.empty(PER, np.float32)
        res_core[order] = cos_sorted
        outs.append(res_core)
    out = np.concatenate(outs).reshape(E_TOTAL, 1).astype(np.float32)
    # keep index inputs' dtype untouched; output is f32 like the reference
    del idx_dtype
    return out



# revision 16
# speedup vs baseline: 5.6032x; 1.0735x over previous
"""CosinePrediction edge-parallel kernel for 8 trn2 NeuronCores.

Strategy: shard the 1M edges across 8 cores (125k each). Per core, bucket
edges host-side by (src_quarter, dst_quarter) of the 100k-row tables —
16 buckets, capacity 8192, padded with index 0 — so row indices fit the
int16 index format of GPSIMD dma_gather. Each bucket does two dma_gather
ops (raw f32 rows of both tables, 4 SWDGE queues round-robin, double
buffered), then on-chip: s_uv = reduce(u*v), s_uu = reduce(u^2) (ACT
square + DVE reduce), s_vv likewise, and a tail computes
cos = s_uv * rsqrt(s_uu*s_vv). The host un-permutes bucket-sorted results.
"""
import sys
import os

sys.path.insert(0, "/opt/trn_rl_repo")

import numpy as np
from contextlib import ExitStack

N = 100_000          # rows per table
D = 64               # feature dim (256B rows)
E_TOTAL = 1_000_000
NCORES = 8
PER = E_TOTAL // NCORES     # 125000 edges per core
NQUART = 4
Q = N // NQUART             # 25000 rows per quarter (< int16 max)
NB = NQUART * NQUART        # 16 buckets per core
CAP = 8192                  # bucket capacity = 128 * 64 (max observed
                            # bucket count is 8019; raises on overflow)
BLK = CAP // 128            # 64 dst blocks
IDXC = CAP // 16            # 512 idx columns (wrapped in 16 partitions)
COLS = NB * BLK             # 1024 result columns

LAST_RESULT = None
_CACHED_NC = None


def _install_trace_shim():
    """Register the NTFF profile hook trn_boot couldn't (stub antenv), and
    neuter the S3 artifact upload. Only needed when BASS_TRACE=1."""
    try:
        import types
        if "antenv.axon_hooks" not in sys.modules:
            from trn_agent_boot.trn_boot import _ntff_profile_via_ctypes
            hook = _ntff_profile_via_ctypes("/opt/axon/libaxon_pjrt.so")
            mod = types.ModuleType("antenv.axon_hooks")
            mod.get_axon_ntff_profile_hook = lambda: hook
            mod.set_axon_ntff_profile_hook = lambda h: None
            sys.modules["antenv.axon_hooks"] = mod
            import antenv
            antenv.axon_hooks = mod
        import concourse.bass_utils as bu
        bu.upload_artifacts = lambda tmpdir: f"file://{tmpdir}"
    except Exception:
        pass


def build_nc():
    global _CACHED_NC
    if _CACHED_NC is not None:
        return _CACHED_NC
    import concourse.bass as bass
    import concourse.bacc as bacc
    import concourse.mybir as mybir
    from concourse.library_config import mlp

    f32 = mybir.dt.float32
    i16 = mybir.dt.int16

    nc = bacc.Bacc("TRN2", target_bir_lowering=False, debug=False,
                   num_swdge_queues=4, detect_race_conditions=False)
    hu = nc.dram_tensor("hu", [N, D], f32, kind="ExternalInput")
    hi = nc.dram_tensor("hi", [N, D], f32, kind="ExternalInput")
    idxu_d = nc.dram_tensor("idxu", [128, NB * IDXC], i16, kind="ExternalInput")
    idxv_d = nc.dram_tensor("idxv", [128, NB * IDXC], i16, kind="ExternalInput")
    out_d = nc.dram_tensor("out", [128, COLS], f32, kind="ExternalOutput")

    with ExitStack() as st:
        u = [st.enter_context(nc.sbuf_tensor(f"u{s}", [128, BLK, D], f32))
             for s in range(2)]
        v = [st.enter_context(nc.sbuf_tensor(f"v{s}", [128, BLK, D], f32))
             for s in range(2)]
        m = [st.enter_context(nc.sbuf_tensor(f"m{s}", [128, BLK, D], f32))
             for s in range(2)]
        w = [st.enter_context(nc.sbuf_tensor(f"w{s}", [128, BLK, D], f32))
             for s in range(2)]
        idxu = st.enter_context(nc.sbuf_tensor("idxu_sb", [128, NB * IDXC], i16))
        idxv = st.enter_context(nc.sbuf_tensor("idxv_sb", [128, NB * IDXC], i16))
        suv = st.enter_context(nc.sbuf_tensor("suv", [128, COLS], f32))
        suu = st.enter_context(nc.sbuf_tensor("suu", [128, COLS], f32))
        svv = st.enter_context(nc.sbuf_tensor("svv", [128, COLS], f32))

        S_idx = st.enter_context(nc.semaphore("S_idx"))
        S_q = [st.enter_context(nc.semaphore(f"S_q{q}")) for q in range(4)]
        S_suv = [st.enter_context(nc.semaphore(f"S_suv{s}")) for s in range(2)]
        S_usq = [st.enter_context(nc.semaphore(f"S_usq{s}")) for s in range(2)]
        S_vsq = [st.enter_context(nc.semaphore(f"S_vsq{s}")) for s in range(2)]
        S_mult = [st.enter_context(nc.semaphore(f"S_mult{s}")) for s in range(2)]
        S_suu = [st.enter_context(nc.semaphore(f"S_suu{s}")) for s in range(2)]
        S_t = st.enter_context(nc.semaphore("S_t"))
        S_sq = st.enter_context(nc.semaphore("S_sq"))
        S_res = st.enter_context(nc.semaphore("S_res"))
        S_out = st.enter_context(nc.semaphore("S_out"))

        block = st.enter_context(nc.Block())

        @block.sync
        def _(sync):
            sync.dma_start(idxu[:], idxu_d[:, :]).then_inc(S_idx, 16)
            sync.dma_start(idxv[:], idxv_d[:, :]).then_inc(S_idx, 16)
            sync.wait_ge(S_res, 1)
            sync.dma_start(out_d[:, :], suv[:]).then_inc(S_out, 16)
            sync.wait_ge(S_out, 16)

        @block.gpsimd
        def _(gpsimd):
            gpsimd.load_library(mlp)
            gpsimd.wait_ge(S_idx, 32)
            for b in range(NB):
                s, k = b % 2, b // 2
                qs, qd = b // NQUART, b % NQUART
                H, HI = CAP // 2, IDXC // 2
                if k >= 1:
                    gpsimd.wait_ge(S_mult[s], k)
                    gpsimd.wait_ge(S_usq[s], k)
                gpsimd.dma_gather(
                    u[s][:, :BLK // 2, :], hu[qs * Q:(qs + 1) * Q, :],
                    idxu[:, b * IDXC:b * IDXC + HI],
                    H, H, D, single_packet=False, queue_num=0,
                ).then_inc(S_q[0], 16)
                gpsimd.dma_gather(
                    u[s][:, BLK // 2:, :], hu[qs * Q:(qs + 1) * Q, :],
                    idxu[:, b * IDXC + HI:(b + 1) * IDXC],
                    H, H, D, single_packet=False, queue_num=1,
                ).then_inc(S_q[1], 16)
                if k >= 1:
                    gpsimd.wait_ge(S_vsq[s], k)
                gpsimd.dma_gather(
                    v[s][:, :BLK // 2, :], hi[qd * Q:(qd + 1) * Q, :],
                    idxv[:, b * IDXC:b * IDXC + HI],
                    H, H, D, single_packet=False, queue_num=2,
                ).then_inc(S_q[2], 16)
                gpsimd.dma_gather(
                    v[s][:, BLK // 2:, :], hi[qd * Q:(qd + 1) * Q, :],
                    idxv[:, b * IDXC + HI:(b + 1) * IDXC],
                    H, H, D, single_packet=False, queue_num=3,
                ).then_inc(S_q[3], 16)

        @block.vector
        def _(vector):
            for b in range(NB):
                s, k = b % 2, b // 2
                cols = slice(b * BLK, (b + 1) * BLK)
                for q in range(4):
                    vector.wait_ge(S_q[q], 16 * (b + 1))
                vector.tensor_tensor(out=m[s][:], in0=u[s][:], in1=v[s][:],
                                     op=mybir.AluOpType.mult
                                     ).then_inc(S_mult[s], 1)
                vector.tensor_reduce(out=suv[:, cols], in_=m[s][:],
                                     axis=mybir.AxisListType.X,
                                     op=mybir.AluOpType.add,
                                     ).then_inc(S_suv[s], 1)
                vector.wait_ge(S_usq[s], k + 1)
                vector.tensor_reduce(out=suu[:, cols], in_=w[s][:],
                                     axis=mybir.AxisListType.X,
                                     op=mybir.AluOpType.add,
                                     ).then_inc(S_suu[s], 1)
                vector.wait_ge(S_vsq[s], k + 1)
                vector.tensor_reduce(out=svv[:, cols], in_=m[s][:],
                                     axis=mybir.AxisListType.X,
                                     op=mybir.AluOpType.add)
            # tail: cos = s_uv * rsqrt(s_uu * s_vv)
            vector.tensor_tensor(out=suu[:], in0=suu[:], in1=svv[:],
                                 op=mybir.AluOpType.mult).then_inc(S_t, 1)
            vector.wait_ge(S_sq, 1)
            vector.reciprocal(out=suu[:], in_=svv[:])
            vector.tensor_tensor(out=suv[:], in0=suv[:], in1=suu[:],
                                 op=mybir.AluOpType.mult).then_inc(S_res, 1)

        @block.scalar
        def _(scalar):
            for b in range(NB):
                s, k = b % 2, b // 2
                scalar.wait_ge(S_q[0], 16 * (b + 1))
                scalar.wait_ge(S_q[1], 16 * (b + 1))
                if k >= 1:
                    scalar.wait_ge(S_suu[s], k)
                scalar.activation(out=w[s][:], in_=u[s][:],
                                  func=mybir.ActivationFunctionType.Square,
                                  ).then_inc(S_usq[s], 1)
                scalar.wait_ge(S_suv[s], k + 1)
                scalar.activation(out=m[s][:], in_=v[s][:],
                                  func=mybir.ActivationFunctionType.Square,
                                  ).then_inc(S_vsq[s], 1)
            scalar.wait_ge(S_t, 1)
            scalar.activation(out=svv[:], in_=suu[:],
                              func=mybir.ActivationFunctionType.Sqrt,
                              ).then_inc(S_sq, 1)

    nc.compile()
    _CACHED_NC = nc
    return nc


def _prep_core(src, dst):
    """Bucket one core's edges; returns (idxu, idxv, order, counts)."""
    qs = src // Q
    qd = dst // Q
    bucket = qs * NQUART + qd
    order = np.argsort(bucket, kind="stable")
    counts = np.bincount(bucket, minlength=NB)
    if counts.max() > CAP:
        raise RuntimeError(f"bucket overflow: {counts.max()} > {CAP}")
    su, du = src[order], dst[order]
    idxu = np.zeros((128, NB * IDXC), np.int16)
    idxv = np.zeros((128, NB * IDXC), np.int16)
    off = 0
    for b in range(NB):
        n = counts[b]
        lu = np.zeros(CAP, np.int64)
        lv = np.zeros(CAP, np.int64)
        lu[:n] = su[off:off + n] - (b // NQUART) * Q
        lv[:n] = du[off:off + n] - (b % NQUART) * Q
        off += n
        wu = lu.reshape(IDXC, 16).T.astype(np.int16)   # i -> (i%16, i//16)
        wv = lv.reshape(IDXC, 16).T.astype(np.int16)
        idxu[:, b * IDXC:(b + 1) * IDXC] = np.tile(wu, (8, 1))
        idxv[:, b * IDXC:(b + 1) * IDXC] = np.tile(wv, (8, 1))
    return idxu, idxv, order, counts


def kernel(h_user, h_item, src_idx, dst_idx):
    global LAST_RESULT
    from concourse.bass_utils import run_bass_kernel_spmd

    if os.environ.get("BASS_TRACE"):
        _install_trace_shim()

    hu = np.ascontiguousarray(np.asarray(h_user, dtype=np.float32))
    hi = np.ascontiguousarray(np.asarray(h_item, dtype=np.float32))
    src = np.asarray(src_idx).astype(np.int64)
    dst = np.asarray(dst_idx).astype(np.int64)
    idx_dtype = np.asarray(src_idx).dtype

    nc = build_nc()

    in_maps, metas = [], []
    for c in range(NCORES):
        s = src[c * PER:(c + 1) * PER]
        d = dst[c * PER:(c + 1) * PER]
        idxu, idxv, order, counts = _prep_core(s, d)
        in_maps.append({"hu": hu, "hi": hi, "idxu": idxu, "idxv": idxv})
        metas.append((order, counts))

    res = run_bass_kernel_spmd(nc, in_maps, core_ids=list(range(NCORES)))
    LAST_RESULT = res

    outs = []
    for c in range(NCORES):
        order, counts = metas[c]
        arr = res.results[c]["out"].reshape(128, NB, BLK)
        # slot i of bucket b lives at [i % 128, b, i // 128]
        arr2 = arr.transpose(1, 2, 0).reshape(NB, CAP)
        cos_sorted = np.concatenate(
            [arr2[b, :counts[b]] for b in range(NB)])
        res_core = np.empty(PER, np.float32)
        res_core[order] = cos_sorted
        outs.append(res_core)
    out = np.concatenate(outs).reshape(E_TOTAL, 1).astype(np.float32)
    # keep index inputs' dtype untouched; output is f32 like the reference
    del idx_dtype
    return out



# revision 17
# speedup vs baseline: 5.6729x; 1.0124x over previous
"""CosinePrediction edge-parallel kernel for 8 trn2 NeuronCores.

Strategy: shard the 1M edges across 8 cores (125k each). Per core, bucket
edges host-side by (src_quarter, dst_quarter) of the 100k-row tables —
16 buckets, capacity 8192, padded with index 0 — so row indices fit the
int16 index format of GPSIMD dma_gather. Each bucket does two dma_gather
ops (raw f32 rows of both tables, 4 SWDGE queues round-robin, double
buffered), then on-chip: s_uv = reduce(u*v), s_uu = reduce(u^2) (ACT
square + DVE reduce), s_vv likewise, and a tail computes
cos = s_uv * rsqrt(s_uu*s_vv). The host un-permutes bucket-sorted results.
"""
import sys
import os

sys.path.insert(0, "/opt/trn_rl_repo")

import numpy as np
from contextlib import ExitStack

N = 100_000          # rows per table
D = 64               # feature dim (256B rows)
E_TOTAL = 1_000_000
NCORES = 8
PER = E_TOTAL // NCORES     # 125000 edges per core
NQUART = 4
Q = N // NQUART             # 25000 rows per quarter (< int16 max)
NB = NQUART * NQUART        # 16 buckets per core
CAP = 8192                  # bucket capacity = 128 * 64 (max observed
                            # bucket count is 8019; raises on overflow)
BLK = CAP // 128            # 64 dst blocks
IDXC = CAP // 16            # 512 idx columns (wrapped in 16 partitions)
COLS = NB * BLK             # 1024 result columns

LAST_RESULT = None
_CACHED_NC = None


def _install_trace_shim():
    """Register the NTFF profile hook trn_boot couldn't (stub antenv), and
    neuter the S3 artifact upload. Only needed when BASS_TRACE=1."""
    try:
        import types
        if "antenv.axon_hooks" not in sys.modules:
            from trn_agent_boot.trn_boot import _ntff_profile_via_ctypes
            hook = _ntff_profile_via_ctypes("/opt/axon/libaxon_pjrt.so")
            mod = types.ModuleType("antenv.axon_hooks")
            mod.get_axon_ntff_profile_hook = lambda: hook
            mod.set_axon_ntff_profile_hook = lambda h: None
            sys.modules["antenv.axon_hooks"] = mod
            import antenv
            antenv.axon_hooks = mod
        import concourse.bass_utils as bu
        bu.upload_artifacts = lambda tmpdir: f"file://{tmpdir}"
    except Exception:
        pass


def build_nc():
    global _CACHED_NC
    if _CACHED_NC is not None:
        return _CACHED_NC
    import concourse.bass as bass
    import concourse.bacc as bacc
    import concourse.mybir as mybir
    from concourse.library_config import mlp

    f32 = mybir.dt.float32
    i16 = mybir.dt.int16

    nc = bacc.Bacc("TRN2", target_bir_lowering=False, debug=False,
                   num_swdge_queues=4, detect_race_conditions=False)
    hu = nc.dram_tensor("hu", [N, D], f32, kind="ExternalInput")
    hi = nc.dram_tensor("hi", [N, D], f32, kind="ExternalInput")
    idxu_d = nc.dram_tensor("idxu", [128, NB * IDXC], i16, kind="ExternalInput")
    idxv_d = nc.dram_tensor("idxv", [128, NB * IDXC], i16, kind="ExternalInput")
    out_d = nc.dram_tensor("out", [128, COLS], f32, kind="ExternalOutput")

    with ExitStack() as st:
        u = [st.enter_context(nc.sbuf_tensor(f"u{s}", [128, BLK, D], f32))
             for s in range(2)]
        v = [st.enter_context(nc.sbuf_tensor(f"v{s}", [128, BLK, D], f32))
             for s in range(2)]
        m = [st.enter_context(nc.sbuf_tensor(f"m{s}", [128, BLK, D], f32))
             for s in range(2)]
        w = [st.enter_context(nc.sbuf_tensor(f"w{s}", [128, BLK, D], f32))
             for s in range(2)]
        idxu = st.enter_context(nc.sbuf_tensor("idxu_sb", [128, NB * IDXC], i16))
        idxv = st.enter_context(nc.sbuf_tensor("idxv_sb", [128, NB * IDXC], i16))
        suv = st.enter_context(nc.sbuf_tensor("suv", [128, COLS], f32))
        suu = st.enter_context(nc.sbuf_tensor("suu", [128, COLS], f32))
        svv = st.enter_context(nc.sbuf_tensor("svv", [128, COLS], f32))

        S_idx = st.enter_context(nc.semaphore("S_idx"))
        S_q = [st.enter_context(nc.semaphore(f"S_q{q}")) for q in range(4)]
        S_suv = [st.enter_context(nc.semaphore(f"S_suv{s}")) for s in range(2)]
        S_usq = [st.enter_context(nc.semaphore(f"S_usq{s}")) for s in range(2)]
        S_vsq = [st.enter_context(nc.semaphore(f"S_vsq{s}")) for s in range(2)]
        S_mult = [st.enter_context(nc.semaphore(f"S_mult{s}")) for s in range(2)]
        S_suu = [st.enter_context(nc.semaphore(f"S_suu{s}")) for s in range(2)]
        S_t = st.enter_context(nc.semaphore("S_t"))
        S_sq = st.enter_context(nc.semaphore("S_sq"))
        S_res = st.enter_context(nc.semaphore("S_res"))
        S_out = st.enter_context(nc.semaphore("S_out"))

        block = st.enter_context(nc.Block())

        @block.sync
        def _(sync):
            sync.dma_start(idxu[:], idxu_d[:, :]).then_inc(S_idx, 16)
            sync.dma_start(idxv[:], idxv_d[:, :]).then_inc(S_idx, 16)
            sync.wait_ge(S_res, 1)
            sync.dma_start(out_d[:, :896], suv[:, :896]).then_inc(S_out, 16)
            sync.wait_ge(S_res, 2)
            sync.dma_start(out_d[:, 896:], suv[:, 896:]).then_inc(S_out, 16)
            sync.wait_ge(S_out, 32)

        @block.gpsimd
        def _(gpsimd):
            gpsimd.load_library(mlp)
            gpsimd.wait_ge(S_idx, 32)
            for b in range(NB):
                s, k = b % 2, b // 2
                qs, qd = b // NQUART, b % NQUART
                H, HI = CAP // 2, IDXC // 2
                if k >= 1:
                    gpsimd.wait_ge(S_mult[s], k)
                    gpsimd.wait_ge(S_usq[s], k)
                gpsimd.dma_gather(
                    u[s][:, :BLK // 2, :], hu[qs * Q:(qs + 1) * Q, :],
                    idxu[:, b * IDXC:b * IDXC + HI],
                    H, H, D, single_packet=False, queue_num=0,
                ).then_inc(S_q[0], 16)
                gpsimd.dma_gather(
                    u[s][:, BLK // 2:, :], hu[qs * Q:(qs + 1) * Q, :],
                    idxu[:, b * IDXC + HI:(b + 1) * IDXC],
                    H, H, D, single_packet=False, queue_num=1,
                ).then_inc(S_q[1], 16)
                if k >= 1:
                    gpsimd.wait_ge(S_vsq[s], k)
                gpsimd.dma_gather(
                    v[s][:, :BLK // 2, :], hi[qd * Q:(qd + 1) * Q, :],
                    idxv[:, b * IDXC:b * IDXC + HI],
                    H, H, D, single_packet=False, queue_num=2,
                ).then_inc(S_q[2], 16)
                gpsimd.dma_gather(
                    v[s][:, BLK // 2:, :], hi[qd * Q:(qd + 1) * Q, :],
                    idxv[:, b * IDXC + HI:(b + 1) * IDXC],
                    H, H, D, single_packet=False, queue_num=3,
                ).then_inc(S_q[3], 16)

        @block.vector
        def _(vector):
            for b in range(NB):
                s, k = b % 2, b // 2
                cols = slice(b * BLK, (b + 1) * BLK)
                for q in range(4):
                    vector.wait_ge(S_q[q], 16 * (b + 1))
                vector.tensor_tensor(out=m[s][:], in0=u[s][:], in1=v[s][:],
                                     op=mybir.AluOpType.mult
                                     ).then_inc(S_mult[s], 1)
                vector.tensor_reduce(out=suv[:, cols], in_=m[s][:],
                                     axis=mybir.AxisListType.X,
                                     op=mybir.AluOpType.add,
                                     ).then_inc(S_suv[s], 1)
                vector.wait_ge(S_usq[s], k + 1)
                vector.tensor_reduce(out=suu[:, cols], in_=w[s][:],
                                     axis=mybir.AxisListType.X,
                                     op=mybir.AluOpType.add,
                                     ).then_inc(S_suu[s], 1)
                vector.wait_ge(S_vsq[s], k + 1)
                vector.tensor_reduce(out=svv[:, cols], in_=m[s][:],
                                     axis=mybir.AxisListType.X,
                                     op=mybir.AluOpType.add)
                # early tail for buckets 0-13 (cols 0:896), pipelined under
                # the remaining gathers; only cols 896: stay on the critical
                # path after bucket 15. Column ranges are disjoint from the
                # b=14,15 reduce outputs.
                if b == 13:
                    vector.tensor_tensor(out=suu[:, :896], in0=suu[:, :896],
                                         in1=svv[:, :896],
                                         op=mybir.AluOpType.mult
                                         ).then_inc(S_t, 1)
                if b == 14:
                    vector.wait_ge(S_sq, 1)
                    vector.reciprocal(out=suu[:, :896], in_=svv[:, :896])
                    vector.tensor_tensor(out=suv[:, :896], in0=suv[:, :896],
                                         in1=suu[:, :896],
                                         op=mybir.AluOpType.mult
                                         ).then_inc(S_res, 1)
            # late tail: cos = s_uv * rsqrt(s_uu * s_vv) for cols 896:
            vector.tensor_tensor(out=suu[:, 896:], in0=suu[:, 896:],
                                 in1=svv[:, 896:],
                                 op=mybir.AluOpType.mult).then_inc(S_t, 1)
            vector.wait_ge(S_sq, 2)
            vector.reciprocal(out=suu[:, 896:], in_=svv[:, 896:])
            vector.tensor_tensor(out=suv[:, 896:], in0=suv[:, 896:],
                                 in1=suu[:, 896:],
                                 op=mybir.AluOpType.mult).then_inc(S_res, 1)

        @block.scalar
        def _(scalar):
            for b in range(NB):
                s, k = b % 2, b // 2
                scalar.wait_ge(S_q[0], 16 * (b + 1))
                scalar.wait_ge(S_q[1], 16 * (b + 1))
                if k >= 1:
                    scalar.wait_ge(S_suu[s], k)
                scalar.activation(out=w[s][:], in_=u[s][:],
                                  func=mybir.ActivationFunctionType.Square,
                                  ).then_inc(S_usq[s], 1)
                scalar.wait_ge(S_suv[s], k + 1)
                scalar.activation(out=m[s][:], in_=v[s][:],
                                  func=mybir.ActivationFunctionType.Square,
                                  ).then_inc(S_vsq[s], 1)
                if b == 13:
                    scalar.wait_ge(S_t, 1)
                    scalar.activation(out=svv[:, :896], in_=suu[:, :896],
                                      func=mybir.ActivationFunctionType.Sqrt,
                                      ).then_inc(S_sq, 1)
            scalar.wait_ge(S_t, 2)
            scalar.activation(out=svv[:, 896:], in_=suu[:, 896:],
                              func=mybir.ActivationFunctionType.Sqrt,
                              ).then_inc(S_sq, 1)

    nc.compile()
    _CACHED_NC = nc
    return nc


def _prep_core(src, dst):
    """Bucket one core's edges; returns (idxu, idxv, order, counts)."""
    qs = src // Q
    qd = dst // Q
    bucket = qs * NQUART + qd
    order = np.argsort(bucket, kind="stable")
    counts = np.bincount(bucket, minlength=NB)
    if counts.max() > CAP:
        raise RuntimeError(f"bucket overflow: {counts.max()} > {CAP}")
    su, du = src[order], dst[order]
    idxu = np.zeros((128, NB * IDXC), np.int16)
    idxv = np.zeros((128, NB * IDXC), np.int16)
    off = 0
    for b in range(NB):
        n = counts[b]
        lu = np.zeros(CAP, np.int64)
        lv = np.zeros(CAP, np.int64)
        lu[:n] = su[off:off + n] - (b // NQUART) * Q
        lv[:n] = du[off:off + n] - (b % NQUART) * Q
        off += n
        wu = lu.reshape(IDXC, 16).T.astype(np.int16)   # i -> (i%16, i//16)
        wv = lv.reshape(IDXC, 16).T.astype(np.int16)
        idxu[:, b * IDXC:(b + 1) * IDXC] = np.tile(wu, (8, 1))
        idxv[:, b * IDXC:(b + 1) * IDXC] = np.tile(wv, (8, 1))
    return idxu, idxv, order, counts


def kernel(h_user, h_item, src_idx, dst_idx):
    global LAST_RESULT
    from concourse.bass_utils import run_bass_kernel_spmd

    if os.environ.get("BASS_TRACE"):
        _install_trace_shim()

    hu = np.ascontiguousarray(np.asarray(h_user, dtype=np.float32))
    hi = np.ascontiguousarray(np.asarray(h_item, dtype=np.float32))
    src = np.asarray(src_idx).astype(np.int64)
    dst = np.asarray(dst_idx).astype(np.int64)
    idx_dtype = np.asarray(src_idx).dtype

    nc = build_nc()

    in_maps, metas = [], []
    for c in range(NCORES):
        s = src[c * PER:(c + 1) * PER]
        d = dst[c * PER:(c + 1) * PER]
        idxu, idxv, order, counts = _prep_core(s, d)
        in_maps.append({"hu": hu, "hi": hi, "idxu": idxu, "idxv": idxv})
        metas.append((order, counts))

    res = run_bass_kernel_spmd(nc, in_maps, core_ids=list(range(NCORES)))
    LAST_RESULT = res

    outs = []
    for c in range(NCORES):
        order, counts = metas[c]
        arr = res.results[c]["out"].reshape(128, NB, BLK)
        # slot i of bucket b lives at [i % 128, b, i // 128]
        arr2 = arr.transpose(1, 2, 0).reshape(NB, CAP)
        cos_sorted = np.concatenate(
            [arr2[b, :counts[b]] for b in range(NB)])
        res_core = np.empty(PER, np.float32)
        res_core[order] = cos_sorted
        outs.append(res_core)
    out = np.concatenate(outs).reshape(E_TOTAL, 1).astype(np.float32)
    # keep index inputs' dtype untouched; output is f32 like the reference
    del idx_dtype
    return out

